# revision 23
# baseline (speedup 1.0000x reference)
"""MixerDiffAttention Trainium2 kernel.

Sharding: 8 cores = 8 head-pairs (tensor parallel over head-pair dim).
Each core processes BOTH batches for its head-pair: the per-core weight
slice (768 qkv cols + 256 gate cols) stays SBUF-resident, and each core
produces the disjoint output slice y[:, :, hp*256:(hp+1)*256].

Per core, per batch:
  Phase 1 (per 128-token tile): qkv+gate matmul (bf16, K=2048
    accumulated); qk copied to SBUF bf16; RMSNorm stats via ACT
    Square+accum; rstd on DVE (Quake rsqrt); RoPE on DVE in bf16 (2x
    mode); PE-transpose rope'd q/k to feature-major [hd, T]; v + raw
    gate to SBUF (gate -> 2*silu via tanh on ACT + one DVE op).
  Phase 2 (per 256-query chunk, diagonal key-pair first): both
    attention variants share one [128, 4, 256] PSUM score tile
    (2 key tiles x 2 variants), one Exp per key-pair covers all four
    slices; causal masking is a 0/1 multiply on the *probs* on the
    otherwise-idle GpSimd engine; AV accumulates probsT @ [v|2|-2/lam]
    so softmax denominators ride along; epilogue fuses diff-combine,
    SiLU gate and group RMSNorm.
"""
import sys
sys.path.insert(0, "/opt/trn_rl_repo")
import numpy as np
import ml_dtypes
import concourse.bass as bass
from concourse import bacc
import concourse.tile as tile
from concourse import mybir
from concourse.bass_utils import run_bass_kernel_spmd

F32 = mybir.dt.float32
BF16 = mybir.dt.bfloat16
AF = mybir.ActivationFunctionType
ALU = mybir.AluOpType

B, T, D, HD = 2, 2048, 2048, 128
KT = D // 128          # 16 contraction tiles
TT = T // 128          # 16 token tiles
CH = 256               # query-chunk width in phase 2
N_CORES = 8
LAMBDA_INIT = 0.8 - 0.6 * float(np.exp(-0.3 * 6))
ONE_MINUS_LI = 1.0 - LAMBDA_INIT
SCALE = float(HD ** -0.5)
EPS = 1e-6

F32R = mybir.dt.float32r
XW_DT = BF16                      # x / W dtype on device
XW_NP = ml_dtypes.bfloat16


def _bcast_mid(ap, n):
    # [P, F] AP -> [P, n, F] with a zero-stride middle dim
    return bass.AP(tensor=ap.tensor, offset=ap.offset,
                   ap=[ap.ap[0], [0, n], *ap.ap[1:]])


def _rsqrt_dve(nc, pool, ss_ap, width, mean_div, tag):
    """rstd = (ss/mean_div + EPS) ** -0.5 entirely on DVE.

    Quake-III bit-trick seed + 2 Newton iterations (~5e-6 rel err); avoids
    ACT Ln/Sqrt so the whole kernel stays inside one ACT table set."""
    I32 = mybir.dt.int32
    ms = pool.tile([128, width], F32, name=tag + "_ms")
    nc.vector.tensor_scalar(out=ms[:], in0=ss_ap, scalar1=1.0 / mean_div,
                            scalar2=EPS, op0=ALU.mult, op1=ALU.add)
    iv = pool.tile([128, width], I32, name=tag + "_iv")
    nc.vector.tensor_scalar(out=iv[:], in0=ms[:].bitcast(I32), scalar1=1,
                            scalar2=None, op0=ALU.logical_shift_right)
    y = pool.tile([128, width], F32, name=tag + "_y")
    nc.vector.tensor_scalar(out=y[:].bitcast(I32), in0=iv[:], scalar1=-1,
                            scalar2=0x5F3759DF, op0=ALU.mult, op1=ALU.add)
    a = pool.tile([128, width], F32, name=tag + "_a")
    u = pool.tile([128, width], F32, name=tag + "_u")
    for _ in range(2):
        nc.vector.tensor_mul(a[:], y[:], y[:])
        nc.vector.tensor_mul(a[:], a[:], ms[:])
        nc.vector.tensor_scalar(out=u[:], in0=a[:], scalar1=-0.5, scalar2=1.5,
                                op0=ALU.mult, op1=ALU.add)
        nc.vector.tensor_mul(y[:], y[:], u[:])
    return y


def build(tt=TT, nb=B):
    nch = tt * 128 // CH
    nc = bacc.Bacc("TRN2", target_bir_lowering=False, debug=False,
                   num_devices=N_CORES)
    # host-packed so every load is one DMA with >=2KB contiguous runs
    xt_d = nc.dram_tensor("xt", [nb, 128, tt, KT, 128], XW_DT,
                          kind="ExternalInput").ap()
    w_d = nc.dram_tensor("wcat", [128, KT, 1024], XW_DT,
                         kind="ExternalInput").ap()
    cos_d = nc.dram_tensor("cos", [128, tt, 64], F32, kind="ExternalInput").ap()
    sin_d = nc.dram_tensor("sin", [128, tt, 64], F32, kind="ExternalInput").ap()
    mask_d = nc.dram_tensor("masks", [128, 2, CH], F32, kind="ExternalInput").ap()
    id_d = nc.dram_tensor("ident", [128, 128], BF16, kind="ExternalInput").ap()
    ones_d = nc.dram_tensor("ones", [128, 4], F32, kind="ExternalInput").ap()
    y_d = nc.dram_tensor("y", [nb, tt * 128, 256], F32, kind="ExternalOutput").ap()

    with tile.TileContext(nc) as tc:
        with tc.tile_pool(name="bigs", bufs=1) as bigs, \
             tc.tile_pool(name="consts", bufs=1) as consts, \
             tc.tile_pool(name="xtp", bufs=3) as xtp:
            # ---- input DMAs: first x tile, weights, consts, more x ----
            xT_pre = {}
            xp0 = xtp.tile([128, KT, 128], XW_DT, name="xT_t")
            nc.sync.dma_start(xp0[:], xt_d[0, :, 0])
            xT_pre[0] = xp0
            wcat = bigs.tile([128, KT, 1024], XW_DT)
            for k in range(KT):
                nc.sync.dma_start(wcat[:, k, :], w_d[:, k, :])
            cos_sb = consts.tile([128, tt, 64], F32)
            nc.sync.dma_start(cos_sb[:], cos_d)
            sin_sb = consts.tile([128, tt, 64], F32)
            nc.sync.dma_start(sin_sb[:], sin_d)
            id_sb = consts.tile([128, 128], BF16)
            nc.sync.dma_start(id_sb[:], id_d)
            ones_sb = consts.tile([128, 4], F32)
            nc.sync.dma_start(ones_sb[:], ones_d)
            mask_sb = consts.tile([128, 2, CH], F32)
            nc.sync.dma_start(mask_sb[:], mask_d)
            for t in (1, 2):
                if t < tt:
                    xp = xtp.tile([128, KT, 128], XW_DT, name="xT_t")
                    nc.sync.dma_start(xp[:], xt_d[0, :, t])
                    xT_pre[t] = xp

            # ---- per-batch persistent (reused sequentially) ----
            qkT = bigs.tile([128, 4, tt * 128], BF16)   # q1,q2,k1,k2 feat-major
            v_sb = bigs.tile([128, tt, 258], F32R)      # [v(256) | 2 | -2/lam]
            g_sb = bigs.tile([128, tt, 256], F32)       # 2*silu(gate)
            # softmax-denominator columns are constant: init once
            for t in range(tt):
                nc.vector.tensor_copy(v_sb[:, t, 256:258], ones_sb[:, 0:2])

            for b in range(nb):
                # ================= Phase 1 =================
                with tc.tile_pool(name="p1t", bufs=2) as p1t, \
                     tc.tile_pool(name="tp_ps", bufs=2, space="PSUM") as tp_ps, \
                     tc.tile_pool(name="mm_ps", bufs=3, space="PSUM") as mm_ps:
                    for t in range(tt):
                        if b == 0 and t in xT_pre:
                            xT_t = xT_pre.pop(t)
                        else:
                            xT_t = xtp.tile([128, KT, 128], XW_DT, name="xT_t")
                            nc.sync.dma_start(xT_t[:], xt_d[b, :, t])
                        qk_ps = mm_ps.tile([128, 512], F32, name="qk_ps")
                        for k in range(KT):
                            nc.tensor.matmul(qk_ps[:], xT_t[:, k, :],
                                             wcat[:, k, 0:512],
                                             start=(k == 0), stop=(k == KT - 1))
                        # qk -> SBUF bf16 early: frees the PSUM bank fast and
                        # lets the RoPE chain run in DVE 2x mode
                        qc = p1t.tile([128, 4, 128], F32, name="qc")
                        nc.scalar.copy(qc[:], qk_ps[:].rearrange(
                            "p (h d) -> p h d", h=4))
                        vg_ps = mm_ps.tile([128, 512], F32, name="vg_ps")
                        for k in range(KT):
                            nc.tensor.matmul(vg_ps[:], xT_t[:, k, :],
                                             wcat[:, k, 512:1024],
                                             start=(k == 0), stop=(k == KT - 1))
                        # ---- q/k rmsnorm stats (ACT) ----
                        ss = p1t.tile([128, 4], F32, name="ss")
                        sq_scr = p1t.tile([128, 128], F32, name="sq_scr")
                        for h in range(4):
                            nc.scalar.activation(sq_scr[:], qc[:, h, :],
                                                 AF.Square, accum_out=ss[:, h:h + 1])
                        rstd = _rsqrt_dve(nc, p1t, ss[:], 4, HD, "rq")
                        # ---- rope (DVE 2x, batched over the 4 head-cols) ----
                        h1, h2 = qc[:, :, 0:64], qc[:, :, 64:128]
                        cos_b = _bcast_mid(cos_sb[:, t, :], 4)
                        sin_b = _bcast_mid(sin_sb[:, t, :], 4)
                        ra = p1t.tile([128, 4, 64], F32, name="ra")
                        rb = p1t.tile([128, 4, 64], F32, name="rb")
                        rot = p1t.tile([128, 4, 128], F32, name="rot")
                        nc.vector.tensor_mul(ra[:], h1, cos_b)
                        nc.vector.tensor_mul(rb[:], h2, sin_b)
                        nc.vector.tensor_add(rot[:, :, 0:64], ra[:], rb[:])
                        nc.vector.tensor_mul(ra[:], h2, cos_b)
                        nc.vector.tensor_mul(rb[:], h1, sin_b)
                        nc.vector.tensor_sub(rot[:, :, 64:128], ra[:], rb[:])
                        qrot = p1t.tile([128, 4, 128], BF16, name="qrot")
                        for h in range(4):
                            nc.vector.tensor_scalar_mul(qrot[:, h, :],
                                                        in0=rot[:, h, :],
                                                        scalar1=rstd[:, h:h + 1])
                        # ---- transpose to feature-major, copy back bf16 ----
                        tq = tp_ps.tile([128, 512], BF16, name="tq")
                        for h in range(4):
                            nc.tensor.transpose(tq[:, h * 128:(h + 1) * 128],
                                                qrot[:, h, :], id_sb[:])
                        nc.scalar.copy(qkT[:, :, t * 128:(t + 1) * 128],
                                       tq[:].rearrange("p (h d) -> p h d", h=4))
                        # ---- v and gate ----
                        nc.vector.tensor_copy(v_sb[:, t, 0:256], vg_ps[:, 0:256])
                        # 2*silu(x) = (tanh(x/2)+1)*x, stays in the exp ACT set
                        th = p1t.tile([128, 256], F32, name="th")
                        nc.scalar.activation(th[:], vg_ps[:, 256:512], AF.Tanh,
                                             scale=0.5)
                        nc.vector.scalar_tensor_tensor(
                            g_sb[:, t, :], th[:], 1.0, vg_ps[:, 256:512],
                            op0=ALU.add, op1=ALU.mult)
                # prefetch next batch's first tiles so phase 1 of b+1 does
                # not queue behind phase 2's output DMAs
                if b + 1 < nb:
                    for t in range(3):
                        xp = xtp.tile([128, KT, 128], XW_DT, name="xT_t")
                        nc.sync.dma_start(xp[:], xt_d[b + 1, :, t])
                        xT_pre[t] = xp
                # ================= Phase 2 =================
                with tc.tile_pool(name="p2s", bufs=11) as p2s, \
                     tc.tile_pool(name="p2e", bufs=3) as p2e, \
                     tc.tile_pool(name="sc_ps", bufs=2, space="PSUM") as sc_ps, \
                     tc.tile_pool(name="av_ps", bufs=4, space="PSUM") as av_ps:

                    def emit_sc(c, jp):
                        # both variants' transposed scores for key tiles
                        # 2jp, 2jp+1 in one 2-bank PSUM tile
                        scp = sc_ps.tile([128, 4, CH], F32, name="scp")
                        for jj in range(2):
                            for var in range(2):
                                nc.tensor.matmul(
                                    scp[:, 2 * jj + var, :],
                                    qkT[:, 2 + var,
                                        (2 * jp + jj) * 128:(2 * jp + jj + 1) * 128],
                                    qkT[:, var, c * CH:(c + 1) * CH],
                                    start=True, stop=True)
                        probs = p2s.tile([128, 4, CH], F32R, name="probs")
                        nc.scalar.activation(probs[:], scp[:], AF.Exp,
                                             scale=SCALE)
                        if jp == c:
                            # causal mask: zero future probs on GpSimd
                            for jj in range(2):
                                nc.gpsimd.tensor_mul(
                                    probs[:, 2 * jj:2 * jj + 2, :],
                                    probs[:, 2 * jj:2 * jj + 2, :],
                                    _bcast_mid(mask_sb[:, jj, :], 2))
                        return probs

                    def emit_av(yacc, jp, probs, first, last):
                        for jj in range(2):
                            j = 2 * jp + jj
                            for var in range(2):
                                for m in range(2):
                                    nc.tensor.matmul(
                                        yacc[(var, m)][:],
                                        probs[:, 2 * jj + var,
                                              m * 128:(m + 1) * 128],
                                        v_sb[:, j, :],
                                        start=(first and jj == 0),
                                        stop=(last and jj == 1))

                    def epilogue(c, yacc):
                        ssy = p2e.tile([128, 2], F32, name="ssy")
                        ygs = []
                        for m in range(2):
                            y1p, y2p = yacc[(0, m)], yacc[(1, m)]
                            # v col 256 = 2 -> 2*s1; col 257 = -2/lam
                            r1 = p2e.tile([128, 1], F32, name="r1")
                            r2n = p2e.tile([128, 1], F32, name="r2n")
                            nc.vector.reciprocal(r1[:], y1p[:, 256:257])
                            nc.vector.reciprocal(r2n[:], y2p[:, 257:258])
                            t1 = p2e.tile([128, 256], F32, name="t1")
                            nc.vector.tensor_scalar_mul(t1[:], in0=y1p[:, 0:256],
                                                        scalar1=r1[:])
                            yt = p2e.tile([128, 256], F32, name="yt")
                            nc.vector.scalar_tensor_tensor(
                                yt[:], y2p[:, 0:256], r2n[:], t1[:],
                                op0=ALU.mult, op1=ALU.add)
                            # g_sb holds 2*silu -> the /2 cancels here
                            yg = p2e.tile([128, 256], F32, name="yg")
                            nc.vector.tensor_mul(yg[:], yt[:],
                                                 g_sb[:, 2 * c + m, :])
                            sq2 = p2e.tile([128, 256], F32, name="sq2")
                            nc.scalar.activation(sq2[:], yg[:], AF.Square,
                                                 accum_out=ssy[:, m:m + 1])
                            ygs.append(yg)
                        rsy = _rsqrt_dve(nc, p2e, ssy[:], 2, 256, "ry")
                        for m in range(2):
                            qt = 2 * c + m
                            out_t = p2e.tile([128, 256], F32, name="out_t")
                            nc.vector.tensor_scalar(
                                out=out_t[:], in0=ygs[m][:],
                                scalar1=rsy[:, m:m + 1], scalar2=ONE_MINUS_LI,
                                op0=ALU.mult, op1=ALU.mult)
                            nc.sync.dma_start(
                                y_d[b, qt * 128:(qt + 1) * 128, :], out_t[:])

                    # Flat cross-chunk pipeline.  Score groups stream in
                    # chunk order with each chunk's DIAGONAL pair first (its
                    # exp + GpSimd mask chain hides behind the chunk); AV
                    # groups lag the score stream by two groups, with each
                    # chunk's diagonal AV last (start/stop flags per chunk).
                    # The lag keeps the PE fed while exps complete, and
                    # chunk boundaries overlap instead of serializing.
                    sc_list = []
                    av_list = []
                    for c in range(nch):
                        sc_list.append((c, c))
                        sc_list.extend((c, jp) for jp in range(c))
                        av_list.extend((c, jp) for jp in range(c))
                        av_list.append((c, c))
                    probs_map = {}
                    yaccs = {}

                    def do_av(idx):
                        c, jp = av_list[idx]
                        if c not in yaccs:
                            yaccs[c] = {
                                (var, m): av_ps.tile([128, 258], F32,
                                                     name="yacc", tag="yacc")
                                for var in range(2) for m in range(2)}
                        emit_av(yaccs[c], jp, probs_map.pop((c, jp)),
                                first=(jp == 0), last=(jp == c))
                        if jp == c:
                            epilogue(c, yaccs.pop(c))

                    for i, (c, jp) in enumerate(sc_list):
                        probs_map[(c, jp)] = emit_sc(c, jp)
                        if i >= 2:
                            do_av(i - 2)
                    do_av(len(av_list) - 2)
                    do_av(len(av_list) - 1)
    nc.compile()
    return nc


_NC = None


def prep_in_maps(hidden_states, W_qkv, lambda_q1, lambda_k1, lambda_q2,
                 lambda_k2, W_g):
    x = np.asarray(hidden_states, dtype=np.float32)
    W_qkv = np.asarray(W_qkv, dtype=np.float32)
    W_g = np.asarray(W_g, dtype=np.float32)

    # xt[b] packed [128, TT, KT, 128]: el (p,tt,k,tau) = x[b, tt*128+tau, k*128+p]
    xt = np.ascontiguousarray(
        x.transpose(0, 2, 1).reshape(B, KT, 128, TT, 128).transpose(0, 2, 3, 1, 4)
    ).astype(XW_NP)

    t_ar = np.arange(T, dtype=np.float32)
    inv_freq = (1.0 / 10000.0 ** (np.arange(0, HD, 2, dtype=np.float32) / HD)
                ).astype(np.float32)
    freqs = np.outer(t_ar, inv_freq).astype(np.float32)
    # [128, TT, 64]: el (p, tt, f) = cos[tt*128+p, f]
    cos = np.ascontiguousarray(
        np.cos(freqs).astype(np.float32).reshape(TT, 128, 64).transpose(1, 0, 2))
    sin = np.ascontiguousarray(
        np.sin(freqs).astype(np.float32).reshape(TT, 128, 64).transpose(1, 0, 2))

    # 0/1 keep-masks multiplied into the diagonal-pair probs
    masks = np.empty((128, 2, CH), dtype=np.float32)
    kk = np.arange(128)[:, None]
    qq = np.arange(CH)[None, :]
    for m in range(2):
        masks[:, m, :] = np.where(m * 128 + kk <= qq, 1.0, 0.0)

    ident = np.eye(128, dtype=np.float32).astype(ml_dtypes.bfloat16)

    lam1 = np.exp(np.sum(np.asarray(lambda_q1, np.float32)
                         * np.asarray(lambda_k1, np.float32), axis=-1))
    lam2 = np.exp(np.sum(np.asarray(lambda_q2, np.float32)
                         * np.asarray(lambda_k2, np.float32), axis=-1))
    lam = (lam1 - lam2 + LAMBDA_INIT).astype(np.float32)   # [8]

    in_maps = []
    for c in range(N_CORES):
        base = 2 * c * 384
        w_cols = [
            W_qkv[:, base:base + 128],            # q1
            W_qkv[:, base + 384:base + 512],      # q2
            W_qkv[:, base + 128:base + 256],      # k1
            W_qkv[:, base + 512:base + 640],      # k2
            W_qkv[:, base + 256:base + 384],      # v1
            W_qkv[:, base + 640:base + 768],      # v2
            W_g[:, c * 256:(c + 1) * 256],        # gate
        ]
        wfull = np.concatenate(w_cols, axis=1)    # [2048, 1024]
        # packed [128, KT, 1024]: el (p,k,c) = w[k*128+p, c]
        wcat = np.ascontiguousarray(
            wfull.reshape(KT, 128, 1024).transpose(1, 0, 2)).astype(XW_NP)
        ones = np.zeros((128, 4), dtype=np.float32)
        ones[:, 0] = 2.0
        ones[:, 1] = -2.0 / lam[c]
        in_maps.append({
            "xt": xt, "wcat": wcat, "cos": cos, "sin": sin,
            "masks": masks, "ident": ident, "ones": ones,
        })

    return in_maps


def kernel(hidden_states, W_qkv, lambda_q1, lambda_k1, lambda_q2, lambda_k2,
           W_g, **run_kwargs):
    global _NC
    if _NC is None:
        _NC = build()
    in_maps = prep_in_maps(hidden_states, W_qkv, lambda_q1, lambda_k1,
                           lambda_q2, lambda_k2, W_g)
    res = run_bass_kernel_spmd(_NC, in_maps, core_ids=list(range(N_CORES)),
                               **run_kwargs)
    out = np.empty((B, T, D), dtype=np.float32)
    for c in range(N_CORES):
        out[:, :, c * 256:(c + 1) * 256] = res.results[c]["y"]
    if run_kwargs:
        return out, res
    return out


# revision 26
# speedup vs baseline: 1.0475x; 1.0475x over previous
"""MixerDiffAttention Trainium2 kernel.

Sharding: 8 cores = 8 head-pairs (tensor parallel over head-pair dim).
Each core processes BOTH batches for its head-pair: the per-core weight
slice (768 qkv cols + 256 gate cols) stays SBUF-resident, and each core
produces the disjoint output slice y[:, :, hp*256:(hp+1)*256].

Per core, per batch:
  Phase 1 (per 128-token tile): qkv+gate matmul (bf16, K=2048
    accumulated); qk copied to SBUF bf16; RMSNorm stats via ACT
    Square+accum; rstd on DVE (Quake rsqrt); RoPE on DVE in bf16 (2x
    mode); PE-transpose rope'd q/k to feature-major [hd, T]; v + raw
    gate to SBUF (gate -> 2*silu via tanh on ACT + one DVE op).
  Phase 2 (per 256-query chunk, diagonal key-pair first): both
    attention variants share one [128, 4, 256] PSUM score tile
    (2 key tiles x 2 variants), one Exp per key-pair covers all four
    slices; causal masking is a 0/1 multiply on the *probs* on the
    otherwise-idle GpSimd engine; AV accumulates probsT @ [v|2|-2/lam]
    so softmax denominators ride along; epilogue fuses diff-combine,
    SiLU gate and group RMSNorm.
"""
import sys
sys.path.insert(0, "/opt/trn_rl_repo")
import numpy as np
import ml_dtypes
import concourse.bass as bass
from concourse import bacc
import concourse.tile as tile
from concourse import mybir
from concourse.bass_utils import run_bass_kernel_spmd

F32 = mybir.dt.float32
BF16 = mybir.dt.bfloat16
AF = mybir.ActivationFunctionType
ALU = mybir.AluOpType

B, T, D, HD = 2, 2048, 2048, 128
KT = D // 128          # 16 contraction tiles
TT = T // 128          # 16 token tiles
CH = 256               # query-chunk width in phase 2
N_CORES = 8
LAMBDA_INIT = 0.8 - 0.6 * float(np.exp(-0.3 * 6))
ONE_MINUS_LI = 1.0 - LAMBDA_INIT
SCALE = float(HD ** -0.5)
EPS = 1e-6

F32R = mybir.dt.float32r
XW_DT = BF16                      # x / W dtype on device
XW_NP = ml_dtypes.bfloat16


def _bcast_mid(ap, n):
    # [P, F] AP -> [P, n, F] with a zero-stride middle dim
    return bass.AP(tensor=ap.tensor, offset=ap.offset,
                   ap=[ap.ap[0], [0, n], *ap.ap[1:]])


def _rsqrt_dve(nc, pool, ss_ap, width, mean_div, tag):
    """rstd = (ss/mean_div + EPS) ** -0.5 entirely on DVE.

    Quake-III bit-trick seed + 2 Newton iterations (~5e-6 rel err); avoids
    ACT Ln/Sqrt so the whole kernel stays inside one ACT table set."""
    I32 = mybir.dt.int32
    ms = pool.tile([128, width], F32, name=tag + "_ms")
    nc.vector.tensor_scalar(out=ms[:], in0=ss_ap, scalar1=1.0 / mean_div,
                            scalar2=EPS, op0=ALU.mult, op1=ALU.add)
    iv = pool.tile([128, width], I32, name=tag + "_iv")
    nc.vector.tensor_scalar(out=iv[:], in0=ms[:].bitcast(I32), scalar1=1,
                            scalar2=None, op0=ALU.logical_shift_right)
    y = pool.tile([128, width], F32, name=tag + "_y")
    nc.vector.tensor_scalar(out=y[:].bitcast(I32), in0=iv[:], scalar1=-1,
                            scalar2=0x5F3759DF, op0=ALU.mult, op1=ALU.add)
    a = pool.tile([128, width], F32, name=tag + "_a")
    u = pool.tile([128, width], F32, name=tag + "_u")
    for _ in range(2):
        nc.vector.tensor_mul(a[:], y[:], y[:])
        nc.vector.tensor_mul(a[:], a[:], ms[:])
        nc.vector.tensor_scalar(out=u[:], in0=a[:], scalar1=-0.5, scalar2=1.5,
                                op0=ALU.mult, op1=ALU.add)
        nc.vector.tensor_mul(y[:], y[:], u[:])
    return y


def build(tt=TT, nb=B):
    nch = tt * 128 // CH
    nc = bacc.Bacc("TRN2", target_bir_lowering=False, debug=False,
                   num_devices=N_CORES)
    # host-packed so every load is one DMA with >=2KB contiguous runs
    xt_d = nc.dram_tensor("xt", [nb, 128, tt, KT, 128], XW_DT,
                          kind="ExternalInput").ap()
    w_d = nc.dram_tensor("wcat", [128, 2, KT, 512], XW_DT,
                         kind="ExternalInput").ap()
    cos_d = nc.dram_tensor("cos", [128, tt, 128], F32, kind="ExternalInput").ap()
    sin_d = nc.dram_tensor("sin", [128, tt, 128], F32, kind="ExternalInput").ap()
    mask_d = nc.dram_tensor("masks", [128, 2, CH], F32, kind="ExternalInput").ap()
    id_d = nc.dram_tensor("ident", [128, 128], BF16, kind="ExternalInput").ap()
    ones_d = nc.dram_tensor("ones", [128, 4], F32, kind="ExternalInput").ap()
    y_d = nc.dram_tensor("y", [nb, tt * 128, 256], F32, kind="ExternalOutput").ap()

    with tile.TileContext(nc) as tc:
        with tc.tile_pool(name="bigs", bufs=1) as bigs, \
             tc.tile_pool(name="consts", bufs=1) as consts, \
             tc.tile_pool(name="xtp", bufs=3) as xtp:
            # ---- input DMAs: first x tile, weights, consts, more x ----
            xT_pre = {}
            xp0 = xtp.tile([128, KT, 128], XW_DT, name="xT_t")
            nc.sync.dma_start(xp0[:], xt_d[0, :, 0])
            xT_pre[0] = xp0
            wcat = bigs.tile([128, 2, KT, 512], XW_DT)
            for g in range(4):
                nc.sync.dma_start(wcat[:, 0, 4 * g:4 * g + 4, :],
                                  w_d[:, 0, 4 * g:4 * g + 4, :])
            for t in (1, 2):
                if t < tt:
                    xp = xtp.tile([128, KT, 128], XW_DT, name="xT_t")
                    nc.sync.dma_start(xp[:], xt_d[0, :, t])
                    xT_pre[t] = xp
            for g in range(4):
                nc.sync.dma_start(wcat[:, 1, 4 * g:4 * g + 4, :],
                                  w_d[:, 1, 4 * g:4 * g + 4, :])
            cos_sb = consts.tile([128, tt, 128], F32)
            nc.sync.dma_start(cos_sb[:], cos_d)
            sin_sb = consts.tile([128, tt, 128], F32)
            nc.sync.dma_start(sin_sb[:], sin_d)
            id_sb = consts.tile([128, 128], BF16)
            nc.sync.dma_start(id_sb[:], id_d)
            ones_sb = consts.tile([128, 4], F32)
            nc.sync.dma_start(ones_sb[:], ones_d)
            mask_sb = consts.tile([128, 2, CH], F32)
            nc.sync.dma_start(mask_sb[:], mask_d)

            # ---- per-batch persistent (reused sequentially) ----
            qkT = bigs.tile([128, 4, tt * 128], BF16)   # q1,q2,k1,k2 feat-major
            v_sb = bigs.tile([128, tt, 258], F32R)      # [v(256) | 2 | -2/lam]
            g_sb = bigs.tile([128, tt, 256], F32)       # 2*silu(gate)
            # softmax-denominator columns are constant: init once
            for t in range(tt):
                nc.vector.tensor_copy(v_sb[:, t, 256:258], ones_sb[:, 0:2])

            for b in range(nb):
                # ================= Phase 1 =================
                with tc.tile_pool(name="p1t", bufs=2) as p1t, \
                     tc.tile_pool(name="tp_ps", bufs=2, space="PSUM") as tp_ps, \
                     tc.tile_pool(name="mm_ps", bufs=3, space="PSUM") as mm_ps:
                    for t in range(tt):
                        if b == 0 and t in xT_pre:
                            xT_t = xT_pre.pop(t)
                        else:
                            xT_t = xtp.tile([128, KT, 128], XW_DT, name="xT_t")
                            nc.sync.dma_start(xT_t[:], xt_d[b, :, t])
                        qk_ps = mm_ps.tile([128, 512], F32, name="qk_ps")
                        for k in range(KT):
                            nc.tensor.matmul(qk_ps[:], xT_t[:, k, :],
                                             wcat[:, 0, k, :],
                                             start=(k == 0), stop=(k == KT - 1))
                        # qk -> SBUF bf16 early: frees the PSUM bank fast and
                        # lets the RoPE chain run in DVE 2x mode
                        qc = p1t.tile([128, 4, 128], F32, name="qc")
                        nc.scalar.copy(qc[:], qk_ps[:].rearrange(
                            "p (h d) -> p h d", h=4))
                        vg_ps = mm_ps.tile([128, 512], F32, name="vg_ps")
                        for k in range(KT):
                            nc.tensor.matmul(vg_ps[:], xT_t[:, k, :],
                                             wcat[:, 1, k, :],
                                             start=(k == 0), stop=(k == KT - 1))
                        # ---- q/k rmsnorm stats (ACT) ----
                        ss = p1t.tile([128, 4], F32, name="ss")
                        sq_scr = p1t.tile([128, 128], F32, name="sq_scr")
                        for h in range(4):
                            nc.scalar.activation(sq_scr[:], qc[:, h, :],
                                                 AF.Square, accum_out=ss[:, h:h + 1])
                        rstd = _rsqrt_dve(nc, p1t, ss[:], 4, HD, "rq")
                        # ---- rope (DVE 2x, batched over the 4 head-cols) ----
                        # rope via concatenated tables:
                        #   ra = qc*(cos|sin) = (h1 cos | h2 sin)
                        #   rb = qc*(sin|cos) = (h1 sin | h2 cos)
                        cos_b = _bcast_mid(cos_sb[:, t, :], 4)
                        sin_b = _bcast_mid(sin_sb[:, t, :], 4)
                        ra = p1t.tile([128, 4, 128], F32, name="ra")
                        rb = p1t.tile([128, 4, 128], F32, name="rb")
                        rot = p1t.tile([128, 4, 128], F32, name="rot")
                        nc.vector.tensor_mul(ra[:], qc[:], cos_b)
                        nc.vector.tensor_mul(rb[:], qc[:], sin_b)
                        nc.vector.tensor_add(rot[:, :, 0:64], ra[:, :, 0:64],
                                             ra[:, :, 64:128])
                        nc.vector.tensor_sub(rot[:, :, 64:128], rb[:, :, 64:128],
                                             rb[:, :, 0:64])
                        qrot = p1t.tile([128, 4, 128], BF16, name="qrot")
                        for h in range(4):
                            nc.vector.tensor_scalar_mul(qrot[:, h, :],
                                                        in0=rot[:, h, :],
                                                        scalar1=rstd[:, h:h + 1])
                        # ---- transpose to feature-major, copy back bf16 ----
                        tq = tp_ps.tile([128, 512], BF16, name="tq")
                        for h in range(4):
                            nc.tensor.transpose(tq[:, h * 128:(h + 1) * 128],
                                                qrot[:, h, :], id_sb[:])
                        nc.scalar.copy(qkT[:, :, t * 128:(t + 1) * 128],
                                       tq[:].rearrange("p (h d) -> p h d", h=4))
                        # ---- v and gate ----
                        nc.vector.tensor_copy(v_sb[:, t, 0:256], vg_ps[:, 0:256])
                        # 2*silu(x) = (tanh(x/2)+1)*x, stays in the exp ACT set
                        th = p1t.tile([128, 256], F32, name="th")
                        nc.scalar.activation(th[:], vg_ps[:, 256:512], AF.Tanh,
                                             scale=0.5)
                        nc.vector.scalar_tensor_tensor(
                            g_sb[:, t, :], th[:], 1.0, vg_ps[:, 256:512],
                            op0=ALU.add, op1=ALU.mult)
                # prefetch next batch's first tiles so phase 1 of b+1 does
                # not queue behind phase 2's output DMAs
                if b + 1 < nb:
                    for t in range(3):
                        xp = xtp.tile([128, KT, 128], XW_DT, name="xT_t")
                        nc.sync.dma_start(xp[:], xt_d[b + 1, :, t])
                        xT_pre[t] = xp
                # ================= Phase 2 =================
                with tc.tile_pool(name="p2s", bufs=11) as p2s, \
                     tc.tile_pool(name="p2e", bufs=3) as p2e, \
                     tc.tile_pool(name="sc_ps", bufs=2, space="PSUM") as sc_ps, \
                     tc.tile_pool(name="av_ps", bufs=4, space="PSUM") as av_ps:

                    def emit_sc(c, jp):
                        # both variants' transposed scores for key tiles
                        # 2jp, 2jp+1 in one 2-bank PSUM tile
                        scp = sc_ps.tile([128, 4, CH], F32, name="scp")
                        for jj in range(2):
                            for var in range(2):
                                nc.tensor.matmul(
                                    scp[:, 2 * jj + var, :],
                                    qkT[:, 2 + var,
                                        (2 * jp + jj) * 128:(2 * jp + jj + 1) * 128],
                                    qkT[:, var, c * CH:(c + 1) * CH],
                                    start=True, stop=True)
                        probs = p2s.tile([128, 4, CH], F32R, name="probs")
                        nc.scalar.activation(probs[:], scp[:], AF.Exp,
                                             scale=SCALE)
                        if jp == c:
                            # causal mask: zero future probs on GpSimd
                            for jj in range(2):
                                nc.gpsimd.tensor_mul(
                                    probs[:, 2 * jj:2 * jj + 2, :],
                                    probs[:, 2 * jj:2 * jj + 2, :],
                                    _bcast_mid(mask_sb[:, jj, :], 2))
                        return probs

                    def emit_av(yacc, jp, probs, first, last):
                        for jj in range(2):
                            j = 2 * jp + jj
                            for var in range(2):
                                for m in range(2):
                                    nc.tensor.matmul(
                                        yacc[(var, m)][:],
                                        probs[:, 2 * jj + var,
                                              m * 128:(m + 1) * 128],
                                        v_sb[:, j, :],
                                        start=(first and jj == 0),
                                        stop=(last and jj == 1))

                    def epilogue(c, yacc):
                        ssy = p2e.tile([128, 2], F32, name="ssy")
                        ygs = []
                        for m in range(2):
                            y1p, y2p = yacc[(0, m)], yacc[(1, m)]
                            # v col 256 = 2 -> 2*s1; col 257 = -2/lam
                            r1 = p2e.tile([128, 1], F32, name="r1")
                            r2n = p2e.tile([128, 1], F32, name="r2n")
                            nc.vector.reciprocal(r1[:], y1p[:, 256:257])
                            nc.vector.reciprocal(r2n[:], y2p[:, 257:258])
                            t1 = p2e.tile([128, 256], F32, name="t1")
                            nc.vector.tensor_scalar_mul(t1[:], in0=y1p[:, 0:256],
                                                        scalar1=r1[:])
                            yt = p2e.tile([128, 256], F32, name="yt")
                            nc.vector.scalar_tensor_tensor(
                                yt[:], y2p[:, 0:256], r2n[:], t1[:],
                                op0=ALU.mult, op1=ALU.add)
                            # g_sb holds 2*silu -> the /2 cancels here
                            yg = p2e.tile([128, 256], F32, name="yg")
                            nc.vector.tensor_mul(yg[:], yt[:],
                                                 g_sb[:, 2 * c + m, :])
                            sq2 = p2e.tile([128, 256], F32, name="sq2")
                            nc.scalar.activation(sq2[:], yg[:], AF.Square,
                                                 accum_out=ssy[:, m:m + 1])
                            ygs.append(yg)
                        rsy = _rsqrt_dve(nc, p2e, ssy[:], 2, 256, "ry")
                        out_t = p2e.tile([128, 2, 256], F32, name="out_t")
                        for m in range(2):
                            nc.vector.tensor_scalar(
                                out=out_t[:, m, :], in0=ygs[m][:],
                                scalar1=rsy[:, m:m + 1], scalar2=ONE_MINUS_LI,
                                op0=ALU.mult, op1=ALU.mult)
                        y_view = y_d[b, c * CH:(c + 1) * CH, :].rearrange(
                            "(two p) f -> p two f", two=2)
                        nc.sync.dma_start(y_view, out_t[:])

                    # Flat cross-chunk pipeline.  Score groups stream in
                    # chunk order with each chunk's DIAGONAL pair first (its
                    # exp + GpSimd mask chain hides behind the chunk); AV
                    # groups lag the score stream by two groups, with each
                    # chunk's diagonal AV last (start/stop flags per chunk).
                    # The lag keeps the PE fed while exps complete, and
                    # chunk boundaries overlap instead of serializing.
                    sc_list = []
                    av_list = []
                    for c in range(nch):
                        sc_list.append((c, c))
                        sc_list.extend((c, jp) for jp in range(c))
                        av_list.extend((c, jp) for jp in range(c))
                        av_list.append((c, c))
                    probs_map = {}
                    yaccs = {}

                    def do_av(idx):
                        c, jp = av_list[idx]
                        if c not in yaccs:
                            yaccs[c] = {
                                (var, m): av_ps.tile([128, 258], F32,
                                                     name="yacc", tag="yacc")
                                for var in range(2) for m in range(2)}
                        emit_av(yaccs[c], jp, probs_map.pop((c, jp)),
                                first=(jp == 0), last=(jp == c))
                        if jp == c:
                            epilogue(c, yaccs.pop(c))

                    for i, (c, jp) in enumerate(sc_list):
                        probs_map[(c, jp)] = emit_sc(c, jp)
                        if i >= 2:
                            do_av(i - 2)
                    do_av(len(av_list) - 2)
                    do_av(len(av_list) - 1)
    nc.compile()
    return nc


_NC = None


def prep_in_maps(hidden_states, W_qkv, lambda_q1, lambda_k1, lambda_q2,
                 lambda_k2, W_g):
    x = np.asarray(hidden_states, dtype=np.float32)
    W_qkv = np.asarray(W_qkv, dtype=np.float32)
    W_g = np.asarray(W_g, dtype=np.float32)

    # xt[b] packed [128, TT, KT, 128]: el (p,tt,k,tau) = x[b, tt*128+tau, k*128+p]
    xt = np.ascontiguousarray(
        x.transpose(0, 2, 1).reshape(B, KT, 128, TT, 128).transpose(0, 2, 3, 1, 4)
    ).astype(XW_NP)

    t_ar = np.arange(T, dtype=np.float32)
    inv_freq = (1.0 / 10000.0 ** (np.arange(0, HD, 2, dtype=np.float32) / HD)
                ).astype(np.float32)
    freqs = np.outer(t_ar, inv_freq).astype(np.float32)
    # [128, TT, 64]: el (p, tt, f) = cos[tt*128+p, f]
    cosv = np.cos(freqs).astype(np.float32).reshape(TT, 128, 64).transpose(1, 0, 2)
    sinv = np.sin(freqs).astype(np.float32).reshape(TT, 128, 64).transpose(1, 0, 2)
    cos = np.ascontiguousarray(np.concatenate([cosv, sinv], axis=2))
    sin = np.ascontiguousarray(np.concatenate([sinv, cosv], axis=2))

    # 0/1 keep-masks multiplied into the diagonal-pair probs
    masks = np.empty((128, 2, CH), dtype=np.float32)
    kk = np.arange(128)[:, None]
    qq = np.arange(CH)[None, :]
    for m in range(2):
        masks[:, m, :] = np.where(m * 128 + kk <= qq, 1.0, 0.0)

    ident = np.eye(128, dtype=np.float32).astype(ml_dtypes.bfloat16)

    lam1 = np.exp(np.sum(np.asarray(lambda_q1, np.float32)
                         * np.asarray(lambda_k1, np.float32), axis=-1))
    lam2 = np.exp(np.sum(np.asarray(lambda_q2, np.float32)
                         * np.asarray(lambda_k2, np.float32), axis=-1))
    lam = (lam1 - lam2 + LAMBDA_INIT).astype(np.float32)   # [8]

    in_maps = []
    for c in range(N_CORES):
        base = 2 * c * 384
        w_cols = [
            W_qkv[:, base:base + 128],            # q1
            W_qkv[:, base + 384:base + 512],      # q2
            W_qkv[:, base + 128:base + 256],      # k1
            W_qkv[:, base + 512:base + 640],      # k2
            W_qkv[:, base + 256:base + 384],      # v1
            W_qkv[:, base + 640:base + 768],      # v2
            W_g[:, c * 256:(c + 1) * 256],        # gate
        ]
        wfull = np.concatenate(w_cols, axis=1)    # [2048, 1024]
        # packed [128, KT, 1024]: el (p,k,c) = w[k*128+p, c]
        wcat = np.ascontiguousarray(
            wfull.reshape(KT, 128, 2, 512).transpose(1, 2, 0, 3)).astype(XW_NP)
        ones = np.zeros((128, 4), dtype=np.float32)
        ones[:, 0] = 2.0
        ones[:, 1] = -2.0 / lam[c]
        in_maps.append({
            "xt": xt, "wcat": wcat, "cos": cos, "sin": sin,
            "masks": masks, "ident": ident, "ones": ones,
        })

    return in_maps


def kernel(hidden_states, W_qkv, lambda_q1, lambda_k1, lambda_q2, lambda_k2,
           W_g, **run_kwargs):
    global _NC
    if _NC is None:
        _NC = build()
    in_maps = prep_in_maps(hidden_states, W_qkv, lambda_q1, lambda_k1,
                           lambda_q2, lambda_k2, W_g)
    res = run_bass_kernel_spmd(_NC, in_maps, core_ids=list(range(N_CORES)),
                               **run_kwargs)
    out = np.empty((B, T, D), dtype=np.float32)
    for c in range(N_CORES):
        out[:, :, c * 256:(c + 1) * 256] = res.results[c]["y"]
    if run_kwargs:
        return out, res
    return out
